# revision 1
# baseline (speedup 1.0000x reference)
"""Trainium2 Bass kernel for AvgSPP (avg-pool 32x32 bins + NN upsample back).

Reference computes, for x[B=16, H=256, W=256, C=64] f32:
    out[b, h, w, c] = mean over the 32x32 spatial bin containing (h, w)
(SCALE=8 bins per axis; half-pixel-center NN indexing with an integer ratio
reduces to bin = idx // 32).

Strategy: pure data parallel over batch (2 samples per core, 8 cores), no
collectives. Per core, per (sample, 128-row h-block, 128-col w-half) chunk:
  1. HWDGE DMA in via nc.sync (SP ring): x chunk -> SBUF [128, 8192]
     (h rows on partitions; 32 KB contiguous per partition)
  2. DVE tensor_reduce over w within each 32-col bin, one op per bin
     column -> [128, 4*64]
  3. PE matmul with a 32x32 block-diagonal ones matrix (pre-scaled by
     1/1024): per-32-row-group sum AND broadcast back to all 128 rows in
     one op -> PSUM [128, 256]
  4. ACT copy with 0-stride broadcast source AP (w-repeat x32) PSUM ->
     SBUF [128, 8192]
  5. HWDGE DMA out via nc.scalar (ACT ring) -> out chunk

The kernel is DMA-bound: 32 MiB in + 32 MiB out per core through the 16
SDMA engines (~27 GB/s each, ~430 GB/s aggregate) gives a ~155 us floor;
measured exec is ~168 us (SDMA engines 96-98% occupied). Both HWDGE rings
(SP for loads, ACT for stores) are used so loads and stores queue
independently. Built on bacc.Bacc + nc.compile(), which legalizes Tile's
multi-wait DMA instructions (walrus accepts at most one wait per DMA).
"""

import sys

for _p in ("/opt/trn_rl_repo", "/opt/pypackages"):
    if _p not in sys.path:
        sys.path.append(_p)

import numpy as np

import concourse.bass as bass
import concourse.mybir as mybir
from concourse import bacc
from concourse.tile import TileContext
from concourse.bass_utils import run_bass_kernel_spmd

B, H, W, C = 16, 256, 256, 64
N_CORES = 8
BPC = B // N_CORES  # samples per core
BIN = 32            # spatial bin edge
PB = 128            # h rows per chunk (SBUF partitions)
WH = 128            # w cols per chunk (max)
NV = WH // BIN      # w bins per chunk (4)
NU = PB // BIN      # h bins per chunk (4)
F32 = mybir.dt.float32


def build_nc():
    from contextlib import ExitStack

    nc = bacc.Bacc()
    x = nc.declare_dram_parameter("x", [BPC, H, W, C], F32, isOutput=False)
    out = nc.declare_dram_parameter("out", [BPC, H, W, C], F32, isOutput=True)

    with TileContext(nc) as tc, ExitStack() as ctx:
        const = ctx.enter_context(tc.tile_pool(name="const", bufs=1))
        inp = ctx.enter_context(tc.tile_pool(name="inp", bufs=3))
        outp = ctx.enter_context(tc.tile_pool(name="outp", bufs=3))
        redp = ctx.enter_context(tc.tile_pool(name="red", bufs=4))
        psum = ctx.enter_context(tc.tile_pool(name="psum", bufs=4, space="PSUM"))

        # Block-diagonal ones (x 1/1024) selector: Bm[k, p] = 1/1024 if k//32 == p//32.
        # matmul(Bm, part): out[p, :] = (1/1024) * sum_{k in p's 32-group} part[k, :]
        # i.e. per-bin h-sum AND h-broadcast in one PE op, pre-scaled to the mean.
        Bm = const.tile([PB, PB], F32)
        nc.vector.memset(Bm[:], 0.0)
        for g in range(NU):
            nc.vector.memset(Bm[g * BIN:(g + 1) * BIN, g * BIN:(g + 1) * BIN],
                             1.0 / (BIN * BIN))

        chunks = [(b, hb, wh * WH, WH)
                  for b in range(BPC)
                  for hb in range(H // PB)
                  for wh in range(W // WH)]

        for b, hb, w0, wn in chunks:
            nv = wn // BIN
            xs = x[b, hb * PB:(hb + 1) * PB, w0:w0 + wn, :]
            tin = inp.tile([PB, WH * C], F32)
            nc.sync.dma_start(tin[:, :wn * C], xs.rearrange("h w c -> h (w c)"))

            # sum over w within each bin: [p, c, w(reduce)] -> [p, c], per v
            part = redp.tile([PB, NV * C], F32)
            for v in range(nv):
                nc.vector.tensor_reduce(
                    part[:, v * C:(v + 1) * C],
                    tin[:, v * BIN * C:(v + 1) * BIN * C]
                    .rearrange("p (w c) -> p c w", w=BIN, c=C),
                    axis=mybir.AxisListType.X,
                    op=mybir.AluOpType.add,
                )

            # h-sum within 32-row groups + broadcast to 128 rows, scaled
            pex = psum.tile([PB, NV * C], F32)
            nc.tensor.matmul(pex[:, :nv * C], Bm[:], part[:, :nv * C],
                             start=True, stop=True)

            # w-broadcast: repeat each bin's 64-channel vector 32x
            tout = outp.tile([PB, WH * C], F32)
            nc.scalar.copy(
                tout[:, :wn * C].rearrange("p (v w c) -> p v w c",
                                           v=nv, w=BIN, c=C),
                pex[:, :nv * C].rearrange("p (v c) -> p v c", v=nv, c=C)
                .unsqueeze(2).broadcast_to([PB, nv, BIN, C]),
            )

            od = out[b, hb * PB:(hb + 1) * PB, w0:w0 + wn, :]
            nc.scalar.dma_start(od.rearrange("h w c -> h (w c)"),
                                tout[:, :wn * C])

    nc.compile()
    return nc


_cached_nc = None


def _get_nc():
    global _cached_nc
    if _cached_nc is None:
        _cached_nc = build_nc()
    return _cached_nc


def _run(x, trace=False):
    nc = _get_nc()
    in_maps = [
        {"x": np.ascontiguousarray(x[i * BPC:(i + 1) * BPC])} for i in range(N_CORES)
    ]
    last_err = None
    for attempt in range(3):
        try:
            res = run_bass_kernel_spmd(
                nc, in_maps, core_ids=list(range(N_CORES)), trace=trace
            )
            break
        except Exception as e:  # transient NRT device errors — retry
            last_err = e
            import time

            time.sleep(2.0 * (attempt + 1))
    else:
        raise last_err
    out = np.concatenate([res.results[i]["out"] for i in range(N_CORES)], axis=0)
    return out, res


def kernel(x):
    x = np.asarray(x, dtype=np.float32)
    assert x.shape == (B, H, W, C), x.shape
    try:  # harmless if BASS_TRACE is unset; avoids a crash if it is set
        _install_profiling()
    except Exception:
        pass
    out, _ = _run(x, trace=False)
    return out


def _install_profiling():
    """Wire up the NTFF profile hook that the container's stub antenv lacks.

    Mirrors trn_agent_boot.trn_boot's hook installation (which degrades
    silently when antenv.axon_hooks is missing). Dev/profiling only — the
    grading path (kernel()) never traces.
    """
    import types

    try:
        from antenv.axon_hooks import get_axon_ntff_profile_hook  # noqa: F401
        return
    except ImportError:
        pass

    import antenv

    mod = types.ModuleType("antenv.axon_hooks")
    holder = {"hook": None}
    mod.set_axon_ntff_profile_hook = lambda h: holder.__setitem__("hook", h)
    mod.get_axon_ntff_profile_hook = lambda: holder["hook"]
    sys.modules["antenv.axon_hooks"] = mod
    antenv.axon_hooks = mod

    from trn_agent_boot.trn_boot import _ntff_profile_via_ctypes

    mod.set_axon_ntff_profile_hook(
        _ntff_profile_via_ctypes("/opt/axon/libaxon_pjrt.so")
    )

    # upload_artifacts pushes the NEFF dir to a remote bucket; no creds in
    # this container, and we only need the local trace files.
    import concourse.bass_utils as bu

    bu.upload_artifacts = lambda tmpdir: f"local://{tmpdir}"


def kernel_timed(x):
    _install_profiling()
    x = np.asarray(x, dtype=np.float32)
    out, res = _run(x, trace=True)
    return out, res



# revision 2
# speedup vs baseline: 1.8006x; 1.8006x over previous
"""Trainium2 Bass kernel for AvgSPP (avg-pool 32x32 bins + NN upsample back).

Reference computes, for x[B=16, H=256, W=256, C=64] f32:
    out[b, h, w, c] = mean over the 32x32 spatial bin containing (h, w)
(SCALE=8 bins per axis; half-pixel-center NN indexing with an integer ratio
reduces to bin = idx // 32).

Strategy: pure data parallel over batch (2 samples per core, 8 cores), no
collectives. The whole kernel is DMA-bound, so all device I/O is fp16
(host converts f32 -> fp16 on the way in, fp16 -> f32 on the way out);
the 2e-2 rel-err budget dwarfs fp16's ~5e-4. Halving the bytes halves
the DMA floor vs the f32 version (~155us -> ~78us per core).

Per core, per (sample, 128-row h-block, 128-col w-half) chunk:
  1. HWDGE DMA in via nc.sync (SP ring): fp16 chunk -> SBUF [128, 8192]
     (h rows on partitions; 16 KB contiguous per partition)
  2. DVE pairwise tree-add over w within each 32-col bin: 5 levels of
     packed tensor_tensor ADDs (innermost 64-ch runs are unit-stride, all
     operands fp16 => DVE 2x perf mode) -> part [128, 4*64] fp16.
     (A single strided tensor_reduce runs at ~2.4 cyc/elem on HW and
     would be the bottleneck; the packed tree is ~4x faster.)
  3. PE matmul with a 32x32 block-diagonal ones matrix (pre-scaled by
     1/1024, fp16): per-32-row-group sum AND broadcast back to all 128
     rows in one op -> PSUM [128, 256] f32
  4. ACT copy with 0-stride broadcast source AP (w-repeat x32) PSUM f32
     -> SBUF fp16 [128, 8192]
  5. HWDGE DMA out via nc.scalar (ACT ring) -> fp16 out chunk

Both HWDGE rings (SP for loads, ACT for stores) are used so loads and
stores queue independently across the SDMA engines.
"""

import sys

for _p in ("/opt/trn_rl_repo", "/opt/pypackages"):
    if _p not in sys.path:
        sys.path.append(_p)

import numpy as np

import concourse.bass as bass
import concourse.mybir as mybir
from concourse import bacc
from concourse.tile import TileContext
from concourse.bass_utils import run_bass_kernel_spmd

B, H, W, C = 16, 256, 256, 64
N_CORES = 8
BPC = B // N_CORES  # samples per core
BIN = 32            # spatial bin edge
PB = 128            # h rows per chunk (SBUF partitions)
WH = 128            # w cols per chunk
NV = WH // BIN      # w bins per chunk (4)
NU = PB // BIN      # h bins per chunk (4)
F32 = mybir.dt.float32
F16 = mybir.dt.float16


def build_nc():
    from contextlib import ExitStack

    nc = bacc.Bacc()
    x = nc.declare_dram_parameter("x", [BPC, H, W, C], F16, isOutput=False)
    out = nc.declare_dram_parameter("out", [BPC, H, W, C], F16, isOutput=True)

    with TileContext(nc) as tc, ExitStack() as ctx:
        const = ctx.enter_context(tc.tile_pool(name="const", bufs=1))
        inp = ctx.enter_context(tc.tile_pool(name="inp", bufs=3))
        outp = ctx.enter_context(tc.tile_pool(name="outp", bufs=3))
        tr1 = ctx.enter_context(tc.tile_pool(name="tr1", bufs=2))
        tr2 = ctx.enter_context(tc.tile_pool(name="tr2", bufs=2))
        tr3 = ctx.enter_context(tc.tile_pool(name="tr3", bufs=2))
        tr4 = ctx.enter_context(tc.tile_pool(name="tr4", bufs=2))
        partp = ctx.enter_context(tc.tile_pool(name="part", bufs=4))
        psum = ctx.enter_context(tc.tile_pool(name="psum", bufs=4, space="PSUM"))

        # Block-diagonal ones (x 1/1024) selector: Bm[k, p] = 1/1024 if
        # k//32 == p//32 (1/1024 = 2^-10 is exact in fp16).
        # matmul(Bm, part): out[p, :] = (1/1024) * sum_{k in p's 32-group} part[k, :]
        # i.e. per-bin h-sum AND h-broadcast in one PE op, pre-scaled to the mean.
        Bm = const.tile([PB, PB], F16)
        nc.vector.memset(Bm[:], 0.0)
        for g in range(NU):
            nc.vector.memset(Bm[g * BIN:(g + 1) * BIN, g * BIN:(g + 1) * BIN],
                             1.0 / (BIN * BIN))

        chunks = [(b, hb, wh) for b in range(BPC)
                  for hb in range(H // PB)
                  for wh in range(W // WH)]

        for b, hb, wh in chunks:
            w0 = wh * WH
            xs = x[b, hb * PB:(hb + 1) * PB, w0:w0 + WH, :]
            tin = inp.tile([PB, WH * C], F16)
            nc.sync.dma_start(tin[:], xs.rearrange("h w c -> h (w c)"))

            # w-reduce within each 32-col bin: 5 levels of pairwise adds.
            # Level k: [p, g(4), w(2m), c] -> [p, g, m, c], all APs keep the
            # innermost 64-ch run packed (fp16, stride 1) for DVE 2x mode.
            t1 = tr1.tile([PB, NV * 16 * C], F16)
            t2 = tr2.tile([PB, NV * 8 * C], F16)
            t3 = tr3.tile([PB, NV * 4 * C], F16)
            t4 = tr4.tile([PB, NV * 2 * C], F16)
            part = partp.tile([PB, NV * C], F16)

            def lvl(dst, src, m):
                # src holds [p, (g, 2m, c)], dst gets [p, (g, m, c)]
                sv = src.rearrange("p (g w c) -> p g w c", g=NV, w=2 * m, c=C)
                dv = dst.rearrange("p (g w c) -> p g w c", g=NV, w=m, c=C)
                nc.vector.tensor_tensor(
                    dv, sv[:, :, 0:m, :], sv[:, :, m:2 * m, :],
                    op=mybir.AluOpType.add,
                )

            lvl(t1, tin, 16)
            lvl(t2, t1, 8)
            lvl(t3, t2, 4)
            lvl(t4, t3, 2)
            lvl(part, t4, 1)

            # h-sum within 32-row groups + broadcast to 128 rows, scaled
            pex = psum.tile([PB, NV * C], F32)
            nc.tensor.matmul(pex[:], Bm[:], part[:], start=True, stop=True)

            # w-broadcast: repeat each bin's 64-channel vector 32x, f32->fp16
            tout = outp.tile([PB, WH * C], F16)
            nc.scalar.copy(
                tout[:].rearrange("p (v w c) -> p v w c", v=NV, w=BIN, c=C),
                pex[:].rearrange("p (v c) -> p v c", v=NV, c=C)
                .unsqueeze(2).broadcast_to([PB, NV, BIN, C]),
            )

            od = out[b, hb * PB:(hb + 1) * PB, w0:w0 + WH, :]
            nc.scalar.dma_start(od.rearrange("h w c -> h (w c)"), tout[:])

    nc.compile()
    return nc


_cached_nc = None


def _get_nc():
    global _cached_nc
    if _cached_nc is None:
        _cached_nc = build_nc()
    return _cached_nc


def _run(x, trace=False):
    nc = _get_nc()
    x16 = x.astype(np.float16)
    in_maps = [
        {"x": np.ascontiguousarray(x16[i * BPC:(i + 1) * BPC])}
        for i in range(N_CORES)
    ]
    last_err = None
    for attempt in range(3):
        try:
            res = run_bass_kernel_spmd(
                nc, in_maps, core_ids=list(range(N_CORES)), trace=trace
            )
            break
        except Exception as e:  # transient NRT device errors — retry
            last_err = e
            import time

            time.sleep(2.0 * (attempt + 1))
    else:
        raise last_err
    out = np.concatenate(
        [res.results[i]["out"] for i in range(N_CORES)], axis=0
    ).astype(np.float32)
    return out, res


def kernel(x):
    x = np.asarray(x, dtype=np.float32)
    assert x.shape == (B, H, W, C), x.shape
    try:  # harmless if BASS_TRACE is unset; avoids a crash if it is set
        _install_profiling()
    except Exception:
        pass
    out, _ = _run(x, trace=False)
    return out


def _install_profiling():
    """Wire up the NTFF profile hook that the container's stub antenv lacks.

    Mirrors trn_agent_boot.trn_boot's hook installation (which degrades
    silently when antenv.axon_hooks is missing). Dev/profiling only — the
    grading path (kernel()) never traces.
    """
    import types

    try:
        from antenv.axon_hooks import get_axon_ntff_profile_hook  # noqa: F401
        return
    except ImportError:
        pass

    import antenv

    mod = types.ModuleType("antenv.axon_hooks")
    holder = {"hook": None}
    mod.set_axon_ntff_profile_hook = lambda h: holder.__setitem__("hook", h)
    mod.get_axon_ntff_profile_hook = lambda: holder["hook"]
    sys.modules["antenv.axon_hooks"] = mod
    antenv.axon_hooks = mod

    from trn_agent_boot.trn_boot import _ntff_profile_via_ctypes

    mod.set_axon_ntff_profile_hook(
        _ntff_profile_via_ctypes("/opt/axon/libaxon_pjrt.so")
    )

    # upload_artifacts pushes the NEFF dir to a remote bucket; no creds in
    # this container, and we only need the local trace files.
    import concourse.bass_utils as bu

    bu.upload_artifacts = lambda tmpdir: f"local://{tmpdir}"


def kernel_timed(x):
    _install_profiling()
    x = np.asarray(x, dtype=np.float32)
    out, res = _run(x, trace=True)
    return out, res
